# revision 11
# baseline (speedup 1.0000x reference)
"""Trainium2 Bass kernel for nn_RandomTimeMask.

Reference semantics (P=0.25):
    off = (pos - start[n, c]) mod L        # circular offset per (n, c) row
    out = where(off >= L//4, x, 0)         # zero a circular window of L//4

Strategy: pure data parallel over batch N across 8 NeuronCores. Per core the
x-shard [16, 12, 32768] f32 is viewed as [384, 16384] (each length-L row is
two stacked halves), processed as 3 partition-blocks x 4 column-tiles of
[128, 4096].

Key observation: the zero run is 8192 long and the keep run 24576 long, both
longer than one 4096-wide tile — so within any (subrow, tile) the mask is
all-ones, all-zeros, or a SINGLE step. The host reduces `starts` to two tiny
per-(partition, tile) tables (scale, bias) and the mask becomes one scalar-
engine activation:

    keep = Sigmoid(scale_p * iota + bias_p)   # saturates to exact 1.0 / 0.0
    y    = x * keep                           # one vector op

so each element makes one pass through ACT and one through DVE, and the
kernel runs at the HBM roofline.
"""
import numpy as np
from contextlib import ExitStack

import concourse.bass as bass
import concourse.mybir as mybir
import concourse.tile as tile
from concourse import bacc
from concourse.bass_utils import run_bass_kernel_spmd

N, C, L = 128, 12, 32768
MASK_LEN = L // 4          # 8192, from P=0.25
NCORES = 8
NSH = N // NCORES          # 16 batches per core
HALVES = 2
H = L // HALVES            # 16384 columns per subrow
R = NSH * C * HALVES       # 384 subrows per core
PART = 128
NBLK = R // PART           # 3 partition blocks
TD = 8192                  # DMA column tile (4 MB per transfer)
NTD = H // TD              # 2 DMA tiles per block-row
T = 4096                   # mask/compute column tile (half a DMA tile)
NT = H // T                # 4 compute tiles per block-row
NK = NBLK * NT             # scale/bias table columns

BIG = float(1 << 20)       # sigmoid saturation scale
MASK_MODE = "act"          # "act" (sigmoid on ScalarE) or "dve" (ts on VectorE)

_CACHE = {}
_IOTA = np.broadcast_to(np.arange(4096, dtype=np.int16), (128, 4096)).copy()


def _build_nc():
    nc = bacc.Bacc("TRN2", target_bir_lowering=False, debug=False,
                   num_devices=NCORES)
    x_in = nc.dram_tensor("x", [R, H], mybir.dt.float32, kind="ExternalInput")
    sa_in = nc.dram_tensor("sa", [PART, NK], mybir.dt.float32,
                           kind="ExternalInput")
    sb_in = nc.dram_tensor("sb", [PART, NK], mybir.dt.float32,
                           kind="ExternalInput")
    io_in = nc.dram_tensor("io", [PART, T], mybir.dt.int16,
                           kind="ExternalInput")
    y_out = nc.dram_tensor("y", [R, H], mybir.dt.float32, kind="ExternalOutput")

    with tile.TileContext(nc) as tc, ExitStack() as ctx:
        const_pool = ctx.enter_context(tc.tile_pool(name="const", bufs=1))
        x_pool = ctx.enter_context(tc.tile_pool(name="xp", bufs=4))
        k_pool = ctx.enter_context(tc.tile_pool(name="kp", bufs=3))

        sa_sb = const_pool.tile([PART, NK], mybir.dt.float32)
        nc.sync.dma_start(sa_sb[:], sa_in[:])
        sb_sb = const_pool.tile([PART, NK], mybir.dt.float32)
        nc.sync.dma_start(sb_sb[:], sb_in[:])
        iota_sb = const_pool.tile([PART, T], mybir.dt.int16)
        nc.sync.dma_start(iota_sb[:], io_in[:])

        for b in range(NBLK):
            for td in range(NTD):
                rows = slice(b * PART, (b + 1) * PART)
                cols = slice(td * TD, (td + 1) * TD)
                xt = x_pool.tile([PART, TD], mybir.dt.float32)
                nc.sync.dma_start(xt[:], x_in[rows, cols])
                for half in range(TD // T):
                    t = td * (TD // T) + half
                    k = b * NT + t
                    hcols = slice(half * T, (half + 1) * T)
                    keep = k_pool.tile([PART, T], mybir.dt.float32)
                    if MASK_MODE == "act":
                        nc.scalar.activation(
                            keep[:], iota_sb[:],
                            mybir.ActivationFunctionType.Sigmoid,
                            bias=sb_sb[:, k:k + 1], scale=sa_sb[:, k:k + 1])
                    else:
                        nc.vector.tensor_scalar(
                            keep[:], iota_sb[:],
                            sa_sb[:, k:k + 1], sb_sb[:, k:k + 1],
                            mybir.AluOpType.mult, mybir.AluOpType.is_ge)
                    nc.vector.tensor_tensor(
                        xt[:, hcols], xt[:, hcols], keep[:],
                        mybir.AluOpType.mult)
                nc.sync.dma_start(y_out[rows, cols], xt[:])
    nc.compile()
    return nc


def _step_tables(starts_c):
    """starts_c [NSH, C] -> (scale, bias) tables [PART, NK] f32.

    For subrow r (orig row j=r//2, half h=r%2) and tile t, the in-tile keep
    set is {cl : lo <= cl < hi} with off0 = (h*H + t*T - s_j) mod L,
    lo = MASK_LEN - off0, hi = L - off0 (hi - lo = 24576 > T, so a single
    boundary at most). Encode as sign/threshold (S, B): keep <=> S*cl >= B.
    """
    s_flat = np.asarray(starts_c).reshape(-1).astype(np.int64)  # [NSH*C]
    scale = np.empty((PART, NK), dtype=np.float32)
    bias = np.empty((PART, NK), dtype=np.float32)
    r = np.arange(R)
    j = r // 2
    h = r % 2
    for t in range(NT):
        off0 = np.mod(h * H + t * T - s_flat[j], L)      # [R]
        lo = MASK_LEN - off0
        hi = L - off0
        S = np.ones(R, dtype=np.float64)
        B = np.empty(R, dtype=np.float64)
        full = (lo <= 0) & (hi >= T)
        step_up = (lo > 0) & (lo < T)
        step_dn = (hi > 0) & (hi < T)
        empty = ~(full | step_up | step_dn)
        B[full] = 0.0
        B[step_up] = lo[step_up]
        S[step_dn] = -1.0
        B[step_dn] = -(hi[step_dn] - 1)
        B[empty] = T + 7.0
        for b in range(NBLK):
            k = b * NT + t
            blk = slice(b * PART, (b + 1) * PART)
            if MASK_MODE == "act":
                scale[:, k] = (S[blk] * BIG).astype(np.float32)
                bias[:, k] = ((0.5 - B[blk]) * BIG).astype(np.float32)
            else:
                scale[:, k] = S[blk].astype(np.float32)
                bias[:, k] = B[blk].astype(np.float32)
    return scale, bias


def _prep_core(x_c, starts_c):
    x2 = np.ascontiguousarray(x_c, dtype=np.float32).reshape(R, H)
    scale, bias = _step_tables(starts_c)
    return {"x": x2, "sa": scale, "sb": bias, "io": _IOTA}


def kernel(x, starts, _trace=False, _trace_kwargs=None):
    x = np.asarray(x)
    starts = np.asarray(starts)
    assert x.shape == (N, C, L) and starts.shape == (N, C)

    if "nc" not in _CACHE:
        _CACHE["nc"] = _build_nc()
    nc = _CACHE["nc"]

    in_maps = [
        _prep_core(x[c * NSH:(c + 1) * NSH], starts[c * NSH:(c + 1) * NSH])
        for c in range(NCORES)
    ]
    res = run_bass_kernel_spmd(
        nc, in_maps, list(range(NCORES)),
        trace=_trace, **(_trace_kwargs or {}),
    )
    kernel.last_result = res

    out = np.empty((N, C, L), dtype=x.dtype)
    for c in range(NCORES):
        out[c * NSH:(c + 1) * NSH] = np.asarray(
            res.results[c]["y"]).reshape(NSH, C, L)
    return out


# revision 15
# speedup vs baseline: 1.0359x; 1.0359x over previous
"""Trainium2 Bass kernel for nn_RandomTimeMask.

Reference semantics (P=0.25):
    off = (pos - start[n, c]) mod L        # circular offset per (n, c) row
    out = where(off >= L//4, x, 0)         # zero a circular window of L//4

Strategy: pure data parallel over batch N across 8 NeuronCores. Per core the
x-shard [16, 12, 32768] f32 is viewed as [384, 16384] (each length-L row is
two stacked halves), processed as 3 partition-blocks. DMA moves 4 MB tiles
[128, 8192] (the per-transfer size that saturates HBM); compute runs on
4096-wide half-tiles for finer pipeline overlap.

Key observation: the zero run is 8192 long and the keep run 24576 long, so
within any 4096-wide (subrow, tile) window the mask is all-ones, all-zeros,
or a SINGLE step. The host reduces `starts` to two tiny per-(partition,
tile) tables (scale, bias) and the mask becomes one scalar-engine
activation:

    keep = Sigmoid(scale_p * iota + bias_p)   # saturates to exact 1.0 / 0.0
    y    = x * keep                           # one vector op

so each element makes one pass through ACT and one through DVE (both far
from their throughput limits), and the kernel runs at the HBM roofline
(~48 MB of traffic per core at ~360 GB/s => ~135 us).
"""
import numpy as np
from contextlib import ExitStack

import concourse.mybir as mybir
import concourse.tile as tile
from concourse import bacc
from concourse.bass_utils import run_bass_kernel_spmd

# run_bass_kernel_spmd(trace=True) imports antenv.axon_hooks, which this
# container's antenv stub lacks; register a no-op fallback so an externally
# set BASS_TRACE cannot crash the run (hook=None -> tracing is skipped).
try:
    import antenv.axon_hooks  # noqa: F401
except ImportError:
    import sys as _sys
    import types as _types
    _m = _types.ModuleType("antenv.axon_hooks")
    _m.get_axon_ntff_profile_hook = lambda: None
    _m.set_axon_ntff_profile_hook = lambda h: None
    _sys.modules["antenv.axon_hooks"] = _m
    import antenv as _antenv
    _antenv.axon_hooks = _m

N, C, L = 128, 12, 32768
MASK_LEN = L // 4          # 8192, from P=0.25
NCORES = 8
NSH = N // NCORES          # 16 batches per core
HALVES = 2
H = L // HALVES            # 16384 columns per subrow
R = NSH * C * HALVES       # 384 subrows per core
PART = 128
NBLK = R // PART           # 3 partition blocks
TD = 8192                  # DMA column tile (4 MB per transfer)
NTD = H // TD              # 2 DMA tiles per block-row
T = 4096                   # mask/compute column tile (half a DMA tile)
NT = H // T                # 4 compute tiles per block-row
NK = NBLK * NT             # scale/bias table columns

BIG = float(1 << 20)       # sigmoid saturation scale
MASK_MODE = "act"          # "act" (sigmoid on ScalarE) or "dve" (ts on VectorE)

_CACHE = {}


def _build_nc():
    nc = bacc.Bacc("TRN2", target_bir_lowering=False, debug=False,
                   num_devices=NCORES)
    x_in = nc.dram_tensor("x", [R, H], mybir.dt.float32, kind="ExternalInput")
    sa_in = nc.dram_tensor("sa", [PART, NK], mybir.dt.float32,
                           kind="ExternalInput")
    sb_in = nc.dram_tensor("sb", [PART, NK], mybir.dt.float32,
                           kind="ExternalInput")
    y_out = nc.dram_tensor("y", [R, H], mybir.dt.float32, kind="ExternalOutput")

    with tile.TileContext(nc) as tc, ExitStack() as ctx:
        const_pool = ctx.enter_context(tc.tile_pool(name="const", bufs=1))
        x_pool = ctx.enter_context(tc.tile_pool(name="xp", bufs=4))
        k_pool = ctx.enter_context(tc.tile_pool(name="kp", bufs=3))

        sa_sb = const_pool.tile([PART, NK], mybir.dt.float32)
        nc.sync.dma_start(sa_sb[:], sa_in[:])
        sb_sb = const_pool.tile([PART, NK], mybir.dt.float32)
        nc.sync.dma_start(sb_sb[:], sb_in[:])
        iota_sb = const_pool.tile([PART, T], mybir.dt.int16)
        nc.gpsimd.iota(iota_sb[:], pattern=[[1, T]], base=0, channel_multiplier=0)

        for b in range(NBLK):
            for td in range(NTD):
                rows = slice(b * PART, (b + 1) * PART)
                cols = slice(td * TD, (td + 1) * TD)
                xt = x_pool.tile([PART, TD], mybir.dt.float32)
                nc.sync.dma_start(xt[:], x_in[rows, cols])
                for half in range(TD // T):
                    t = td * (TD // T) + half
                    k = b * NT + t
                    hcols = slice(half * T, (half + 1) * T)
                    keep = k_pool.tile([PART, T], mybir.dt.float32)
                    if MASK_MODE == "act":
                        nc.scalar.activation(
                            keep[:], iota_sb[:],
                            mybir.ActivationFunctionType.Sigmoid,
                            bias=sb_sb[:, k:k + 1], scale=sa_sb[:, k:k + 1])
                    else:
                        nc.vector.tensor_scalar(
                            keep[:], iota_sb[:],
                            sa_sb[:, k:k + 1], sb_sb[:, k:k + 1],
                            mybir.AluOpType.mult, mybir.AluOpType.is_ge)
                    nc.vector.tensor_tensor(
                        xt[:, hcols], xt[:, hcols], keep[:],
                        mybir.AluOpType.mult)
                # out-DMAs go on the ACT HWDGE ring so they never queue
                # behind in-DMAs on the SP ring
                nc.scalar.dma_start(y_out[rows, cols], xt[:])
    nc.compile()
    return nc


def _step_tables(starts_c):
    """starts_c [NSH, C] -> (scale, bias) tables [PART, NK] f32.

    For subrow r (orig row j=r//2, half h=r%2) and tile t, the in-tile keep
    set is {cl : lo <= cl < hi} with off0 = (h*H + t*T - s_j) mod L,
    lo = MASK_LEN - off0, hi = L - off0 (hi - lo = 24576 > T, so a single
    boundary at most). Encode as sign/threshold (S, B): keep <=> S*cl >= B.
    """
    s_flat = np.asarray(starts_c).reshape(-1).astype(np.int64)  # [NSH*C]
    scale = np.empty((PART, NK), dtype=np.float32)
    bias = np.empty((PART, NK), dtype=np.float32)
    r = np.arange(R)
    j = r // 2
    h = r % 2
    for t in range(NT):
        off0 = np.mod(h * H + t * T - s_flat[j], L)      # [R]
        lo = MASK_LEN - off0
        hi = L - off0
        S = np.ones(R, dtype=np.float64)
        B = np.empty(R, dtype=np.float64)
        full = (lo <= 0) & (hi >= T)
        step_up = (lo > 0) & (lo < T)
        step_dn = (hi > 0) & (hi < T)
        empty = ~(full | step_up | step_dn)
        B[full] = 0.0
        B[step_up] = lo[step_up]
        S[step_dn] = -1.0
        B[step_dn] = -(hi[step_dn] - 1)
        B[empty] = T + 7.0
        for b in range(NBLK):
            k = b * NT + t
            blk = slice(b * PART, (b + 1) * PART)
            if MASK_MODE == "act":
                scale[:, k] = (S[blk] * BIG).astype(np.float32)
                bias[:, k] = ((0.5 - B[blk]) * BIG).astype(np.float32)
            else:
                scale[:, k] = S[blk].astype(np.float32)
                bias[:, k] = B[blk].astype(np.float32)
    return scale, bias


def _prep_core(x_c, starts_c):
    x2 = np.ascontiguousarray(x_c, dtype=np.float32).reshape(R, H)
    scale, bias = _step_tables(starts_c)
    return {"x": x2, "sa": scale, "sb": bias}


def kernel(x, starts, _trace=False, _trace_kwargs=None):
    x = np.asarray(x)
    starts = np.asarray(starts)
    assert x.shape == (N, C, L) and starts.shape == (N, C)

    if "nc" not in _CACHE:
        _CACHE["nc"] = _build_nc()
    nc = _CACHE["nc"]

    in_maps = [
        _prep_core(x[c * NSH:(c + 1) * NSH], starts[c * NSH:(c + 1) * NSH])
        for c in range(NCORES)
    ]
    res = run_bass_kernel_spmd(
        nc, in_maps, list(range(NCORES)),
        trace=_trace, **(_trace_kwargs or {}),
    )
    kernel.last_result = res

    out = np.empty((N, C, L), dtype=x.dtype)
    for c in range(NCORES):
        out[c * NSH:(c + 1) * NSH] = np.asarray(
            res.results[c]["y"]).reshape(NSH, C, L)
    return out
